# revision 58
# baseline (speedup 1.0000x reference)
"""Trainium2 Bass kernel for nn_DegModel (EDSR-style degradation backbone +
per-pixel KPN), distributed over 8 NeuronCores.

Sharding: one core per (batch, image-half): core i -> batch i//2, half i%2.
Each core runs the whole backbone locally on its 64-row half plus a 17-row
recomputed halo, so no collectives are needed. Bottom halves are processed
vertically flipped (host flips z, the dy axis of the conv weights, and the
t-axis order of w_out's 441 kernel rows), which makes the on-device geometry
identical for all cores. The only cross-core quantity - the global mean of
the predicted noise channel - is reduced on host from per-core partial sums.

Feature maps live in SBUF as [128 partitions, J slots, 130] with partition
p = channel + 64*parity and the odd-row half skewed one slot down. A 3x3
conv over an 8-row output block is 6 full K=128 x M=128 float32r matmuls
into one [128, 4, 128] PSUM bank (proven to stream back-to-back at
~1 row/cycle regardless of LDWEIGHTS dedup).

KPN patches are only expanded HORIZONTALLY on host: xe[j, c, r, u] =
xpad[c, r, 4j+u] (bf16, [128, 3, 273, 21], 4.4 MB/core vs 21.7 MB fully
expanded). The 21 vertical taps of an output row are then 21 consecutive
r-slices, so the patch tile for row yl is the contiguous SBUF view
xe[:, c, 4*yl : 4*yl+21, :]. xe is prefetched during the backbone.

Per (8-row band, channel) the KPN MAC runs on DVE as one bf16 2x-mode
multiply over a 448-padded overlapping strided view of xe, a 3-level
2x-mode in-place add tree (448->224->112->56) and one small TensorReduce:
pc = sum_tap(exp(logit) * patch); softmax normalization is folded to the
end (y = pc / sum(exp), denominator via the exp's accum_out).

The last BAND_RB ResBlocks + the 1x1 conv/softmax/KPN run as a lag-1.5
wavefront (half-tick schedule: layer d emits band b at tau = 3d + 2b) so
the DVE-heavy KPN overlaps part of the Tensor-heavy backbone drain.
"""

import sys

sys.path.insert(0, "/opt/trn_rl_repo")

import numpy as np

import concourse.bass as bass
import concourse.tile as tile
from concourse import mybir
from concourse.bass_utils import run_bass_kernel_spmd

KSIZE = 21
NF = 64
NB = 8
IN_NC = 3
B, H, W = 4, 512, 512
h = w = 128
NCH = KSIZE * KSIZE + 1  # 442
KK = KSIZE * KSIZE       # 441

N_CORES = 8
J = 44    # feature-buffer slots (2 image rows per slot)
X = 130   # 128 cols + 2 zero pad cols
NMID = 2 * NB
XR = 274  # xe rows per core (4*63 + 21, plus one zero pad row)
KP = 448  # padded tap count for the power-of-2 reduction tree

BAND_RB = 8   # trailing ResBlocks run banded/wavefronted with the KPN
WAVE = "lag15"  # 'lag2' (step-wise, proven) | 'lag15' (half-tick lag-1.5)

_cache = {}


def _enable_ldw_opt():
    import concourse.bass_utils as _bu
    if getattr(_bu, "_ldw_opt_patched", False):
        return
    _orig = _bu.run_command

    def _patched(cmd, **kw):
        if isinstance(cmd, list):
            cmd = ["--enable-ldw-opt=true" if c == "--enable-ldw-opt=false"
                   else c for c in cmd]
        return _orig(cmd, **kw)

    _bu.run_command = _patched
    _bu._ldw_opt_patched = True


def _legalize_waits(nc):
    """This walrus build rejects >1 sync wait per instruction; move extra
    waits onto same-engine NOPs inserted immediately before (engines are
    in-order, so semantics are preserved)."""
    for fn in nc.m.functions:
        for blk in fn.blocks:
            out, changed = [], False
            for inst in blk.instructions:
                si = inst.sync_info
                if si is not None and len(si.on_wait) > 1:
                    waits = list(si.on_wait)
                    for wt in waits[:-1]:
                        nop = mybir.InstNoOp(
                            name=nc.get_next_instruction_name(),
                            ins=[], outs=[], engine=inst.engine)
                        nop.sync_info = mybir.SyncInfo(on_wait=[wt], on_update=[])
                        out.append(nop)
                        changed = True
                    inst.sync_info = mybir.SyncInfo(
                        on_wait=[waits[-1]], on_update=list(si.on_update))
                out.append(inst)
            if changed:
                blk.instructions = out


def _conv_blocks(k_halo):
    hi = (64 + k_halo) // 2 + 1      # top slot of even output rows
    return [(s, min(4, hi - s + 1)) for s in range(1, hi + 1, 4)]


def _build_nc(bout_zero):
    f32 = mybir.dt.float32
    f32r = mybir.dt.float32r
    bf16 = mybir.dt.bfloat16
    relu = mybir.ActivationFunctionType.Relu
    ident = mybir.ActivationFunctionType.Identity
    Aop = mybir.AluOpType
    nc = bass.Bass()

    zg_e = nc.dram_tensor("zg_e", [IN_NC, J, X], f32r, kind="ExternalInput")
    zg_o = nc.dram_tensor("zg_o", [IN_NC, J, X], f32r, kind="ExternalInput")
    wl1_in = nc.dram_tensor("wl1_in", [128, 3, 128], f32r, kind="ExternalInput")
    wl2_in = nc.dram_tensor("wl2_in", [128, 3, 128], f32r, kind="ExternalInput")
    # all mid-layer weights, partition-major for a single big prefetch DMA
    w1m_d = nc.dram_tensor("w1m", [128, NMID, 3, 128], f32r,
                           kind="ExternalInput")
    w2m_d = nc.dram_tensor("w2m", [128, NMID, 3, 128], f32r,
                           kind="ExternalInput")
    wout_lo = nc.dram_tensor("wout_lo", [128, NCH], f32r, kind="ExternalInput")
    wout_hi = nc.dram_tensor("wout_hi", [128, NCH], f32r, kind="ExternalInput")
    biases = nc.dram_tensor("biases", [128, NMID + 1], f32,
                            kind="ExternalInput")
    bout_r = nc.dram_tensor("bout_r", [1, NCH], f32r, kind="ExternalInput")
    ones_r = nc.dram_tensor("ones_r", [1, 128], f32r, kind="ExternalInput")
    # horizontally-expanded KPN patch rows: xe[j, c, r, u] = xpad[c, r, 4j+u]
    xe_d = nc.dram_tensor("xe", [128, IN_NC, XR, KSIZE], bf16,
                          kind="ExternalInput")

    ydev = nc.dram_tensor("ydev", [128, IN_NC, 64], f32, kind="ExternalOutput")
    nsdev = nc.dram_tensor("nsdev", [128, 64], f32, kind="ExternalOutput")

    with tile.TileContext(nc) as tc:
        wpool = tc.alloc_tile_pool(name="w", bufs=1)
        xpool = tc.alloc_tile_pool(name="xe", bufs=1)
        gpool = tc.alloc_tile_pool(name="g", bufs=1)
        epool = tc.alloc_tile_pool(name="exp", bufs=2)
        dpool = tc.alloc_tile_pool(name="dum", bufs=2)
        tpool = tc.alloc_tile_pool(name="rtmp", bufs=3)
        spool = tc.alloc_tile_pool(name="small", bufs=4)
        pcpool = tc.alloc_tile_pool(name="pc", bufs=8)
        psum = tc.alloc_tile_pool(name="ps", bufs=6, space="PSUM")
        psum_o = tc.alloc_tile_pool(name="pso", bufs=2, space="PSUM")

        l1_in = wpool.tile([128, 3, 128], f32r)
        l2_in = wpool.tile([128, 3, 128], f32r)
        w1m = wpool.tile([128, NMID, 3, 128], f32r)
        w2m = wpool.tile([128, NMID, 3, 128], f32r)
        wo_lo = wpool.tile([128, NCH], f32r)
        wo_hi = wpool.tile([128, NCH], f32r)
        bias_t = wpool.tile([128, NMID + 1], f32)
        bo_t = wpool.tile([1, NCH], f32r)
        ones_t = wpool.tile([1, 128], f32r)
        xe_t = xpool.tile([128, IN_NC, XR, KSIZE], bf16)

        g_z = gpool.tile([128, J, X], f32r)
        feat = gpool.tile([128, J, X], f32r)
        t1 = gpool.tile([128, J, X], f32r)
        # All DMAs ride the single sync (SP) queue in priority order: the SP
        # engine is in-order, so the zg issues (gated on the g_z memsets)
        # hold the big prefetches out of the shared DMA engines until the
        # small backbone inputs have landed. The z-input arrays arrive with
        # zero borders baked in and cover full [3, J, X] partition rows (3
        # fat descriptors each).
        half = J // 2
        nc.vector.memset(g_z[:, 0:half].bitcast(f32), 0.0)
        nc.gpsimd.memset(g_z[:, half:J].bitcast(f32), 0.0)
        nc.vector.memset(feat[:, 0:half].bitcast(f32), 0.0)
        nc.gpsimd.memset(feat[:, half:J].bitcast(f32), 0.0)

        nc.sync.dma_start(out=l1_in, in_=wl1_in[:])
        nc.sync.dma_start(out=l2_in, in_=wl2_in[:])
        nc.sync.dma_start(out=bias_t, in_=biases[:])
        nc.sync.dma_start(out=g_z[0:IN_NC], in_=zg_e[:])
        nc.sync.dma_start(out=g_z[64:64 + IN_NC], in_=zg_o[:])
        nc.sync.dma_start(out=w1m[:, 0:2], in_=w1m_d[:, 0:2])
        nc.sync.dma_start(out=w2m[:, 0:2], in_=w2m_d[:, 0:2])
        nc.sync.dma_start(out=w1m[:, 2:NMID], in_=w1m_d[:, 2:NMID])
        nc.sync.dma_start(out=w2m[:, 2:NMID], in_=w2m_d[:, 2:NMID])
        nc.sync.dma_start(out=xe_t, in_=xe_d[:])
        nc.sync.dma_start(out=wo_lo, in_=wout_lo[:])
        nc.sync.dma_start(out=wo_hi, in_=wout_hi[:])
        nc.sync.dma_start(out=bo_t, in_=bout_r[:])
        nc.sync.dma_start(out=ones_t, in_=ones_r[:])
        nc.vector.memset(t1[:, 0:half].bitcast(f32), 0.0)
        nc.gpsimd.memset(t1[:, half:J].bitcast(f32), 0.0)

        yacc = spool.tile([128, IN_NC, 64], f32, tag="yacc")
        nsacc = spool.tile([128, 64], f32, tag="nsacc")

        def conv_block(src, dst, l1, l2, bias_col, func, s0, mc, residual):
            """One 8-row (4-slot) output block of a 3x3 conv."""
            P = psum.tile([128, 4, 128], f32, tag="convps")
            for wi in range(6):
                dx, phase = wi % 3, wi // 3
                wt = (l1 if phase == 0 else l2)[:, dx]
                o = s0 + phase
                nc.tensor.matmul(
                    P[:, 0:mc], wt,
                    src[0:128, o:o + mc, dx:dx + 128],
                    start=(wi == 0), stop=(wi == 5))
            if residual is None:
                nc.scalar.activation(
                    out=dst[0:64, s0:s0 + mc, 1:129],
                    in_=P[0:64, 0:mc],
                    func=func, bias=bias_col[0:64], scale=1.0)
                nc.scalar.activation(
                    out=dst[64:128, s0 + 1:s0 + 1 + mc, 1:129],
                    in_=P[64:128, 0:mc],
                    func=func, bias=bias_col[64:128], scale=1.0)
            else:
                # even half: dst = (P + bias) + residual straight from PSUM
                # on DVE; odd half: ACT identity+bias to a temp, then the
                # residual add on (otherwise idle) GpSimd.
                nc.vector.scalar_tensor_tensor(
                    out=dst[0:64, s0:s0 + mc, 1:129],
                    in0=P[0:64, 0:mc], scalar=bias_col[0:64],
                    in1=residual[0:64, s0:s0 + mc, 1:129],
                    op0=Aop.add, op1=Aop.add)
                tmp = tpool.tile([128, 4, 128], f32, tag="rtmp")
                nc.scalar.activation(
                    out=tmp[64:128, 0:mc], in_=P[64:128, 0:mc],
                    func=ident, bias=bias_col[64:128], scale=1.0)
                nc.gpsimd.tensor_tensor(
                    out=dst[64:128, s0 + 1:s0 + 1 + mc, 1:129],
                    in0=tmp[64:128, 0:mc],
                    in1=residual[64:128, s0 + 1:s0 + 1 + mc, 1:129],
                    op=Aop.add)

        def conv_layer(src, dst, l1, l2, bias_col, func, k_halo, residual):
            for s0, mc in _conv_blocks(k_halo):
                conv_block(src, dst, l1, l2, bias_col, func, s0, mc, residual)

        def kpn_band(bi):
            """KPN for rows y0 = 8*bi .. 8*bi+7."""
            y0 = 8 * bi
            ex2 = epool.tile([128, 8, KP], bf16, tag="ex")
            ssum2 = spool.tile([128, 8], f32, tag="ssum")
            for r in range(8):
                yl = y0 + r
                if yl % 2 == 0:
                    slot, wsel = yl // 2 + 1, wo_lo
                else:
                    slot, wsel = (yl + 1) // 2 + 1, wo_hi
                Po = psum_o.tile([128, NCH], f32, tag="pout")
                nc.tensor.matmul(Po, feat[:, slot, 1:129], wsel,
                                 start=True, stop=bout_zero)
                if not bout_zero:
                    nc.tensor.matmul(Po, ones_t, bo_t, start=False, stop=True)
                nc.scalar.activation(out=ex2[:, r, 0:NCH], in_=Po,
                                     func=mybir.ActivationFunctionType.Exp,
                                     scale=1.0, accum_out=ssum2[:, r:r + 1])
            rcp2 = spool.tile([128, 8], f32, tag="rcp")
            nc.vector.reciprocal(out=rcp2, in_=ssum2)
            # noise channel out, then zero taps 441..447 so the padded
            # 448-wide reduction tree sums exactly the 441 kernel taps.
            nc.gpsimd.tensor_tensor(out=nsacc[:, y0:y0 + 8],
                                    in0=ex2[:, :, NCH - 1], in1=rcp2,
                                    op=Aop.mult)
            nc.vector.memset(ex2[:, :, KK:KP], 0.0)
            for c in range(IN_NC):
                patch8 = bass.AP(
                    tensor=xe_t.tensor,
                    offset=xe_t.offset + (c * XR + 4 * y0) * KSIZE,
                    ap=[list(xe_t.ap[0]), [4 * KSIZE, 8], [1, KP]])
                prod = dpool.tile([128, 8, KP], bf16, tag="prod")
                pc = pcpool.tile([128, 8, 1], f32, tag="pc")
                nc.vector.tensor_tensor(out=prod, in0=ex2, in1=patch8,
                                        op=Aop.mult)
                nc.vector.tensor_tensor(out=prod[:, :, 0:224],
                                        in0=prod[:, :, 0:224],
                                        in1=prod[:, :, 224:448], op=Aop.add)
                nc.vector.tensor_tensor(out=prod[:, :, 0:112],
                                        in0=prod[:, :, 0:112],
                                        in1=prod[:, :, 112:224], op=Aop.add)
                nc.vector.tensor_tensor(out=prod[:, :, 0:56],
                                        in0=prod[:, :, 0:56],
                                        in1=prod[:, :, 56:112], op=Aop.add)
                nc.vector.tensor_reduce(out=pc, in_=prod[:, :, 0:56],
                                        op=Aop.add, axis=mybir.AxisListType.X)
                eng = nc.gpsimd if c == 2 else nc.vector
                eng.tensor_tensor(out=yacc[:, c, y0:y0 + 8], in0=pc[:, :, 0],
                                  in1=rcp2, op=Aop.mult)

        # ---- serial backbone: conv_in + ResBlocks 0 .. NB-BAND_RB-1 ----
        conv_layer(g_z, feat, l1_in, l2_in, bias_t[:, 0:1], ident, 16, None)
        n_serial = NB - BAND_RB
        for rb in range(n_serial):
            la, lb = 2 * rb, 2 * rb + 1
            conv_layer(feat, t1, w1m[:, la], w2m[:, la],
                       bias_t[:, 1 + la:2 + la], relu, 15 - 2 * rb, None)
            conv_layer(t1, feat, w1m[:, lb], w2m[:, lb],
                       bias_t[:, 1 + lb:2 + lb], ident, 14 - 2 * rb, feat)

        # ---- banded tail: last BAND_RB ResBlocks + KPN, lag-2 wavefront ----
        layers = []
        for rb in range(n_serial, NB):
            la, lb = 2 * rb, 2 * rb + 1
            layers.append((feat, t1, w1m[:, la], w2m[:, la],
                           bias_t[:, 1 + la:2 + la], relu, 15 - 2 * rb, None))
            layers.append((t1, feat, w1m[:, lb], w2m[:, lb],
                           bias_t[:, 1 + lb:2 + lb], ident, 14 - 2 * rb, feat))
        nlay = len(layers)
        if nlay == 0:
            for bi in range(8):
                kpn_band(bi)
        elif WAVE == "lag15":
            # lag-1.5 wavefront in half-ticks: layer d emits band b at
            # tau = 3d + 2b; the KPN acts as depth nlay. (ell, b) depends on
            # (ell-1, b+1), emitted exactly one half-tick earlier, so every
            # dependency has a few conv-blocks of pipeline slack.
            blocks_per = [_conv_blocks(lay[6]) for lay in layers]
            tau_max = max(3 * nlay + 2 * 7,
                          max(3 * d + 2 * (len(blocks_per[d]) - 1)
                              for d in range(nlay)))
            for tau in range(tau_max + 1):
                rk = tau - 3 * nlay
                if rk >= 0 and rk % 2 == 0 and rk // 2 < 8:
                    kpn_band(rk // 2)
                for d in range(nlay - 1, -1, -1):
                    r = tau - 3 * d
                    if r >= 0 and r % 2 == 0 and r // 2 < len(blocks_per[d]):
                        src, dst, l1, l2, bias_col, func, _, resid = layers[d]
                        s0, mc = blocks_per[d][r // 2]
                        conv_block(src, dst, l1, l2, bias_col, func,
                                   s0, mc, resid)
        else:
            blocks_per = [_conv_blocks(lay[6]) for lay in layers]
            n_steps = 2 * nlay + 8
            for step in range(n_steps):
                kb = step - 2 * nlay
                if 0 <= kb < 8:
                    kpn_band(kb)
                for d in range(nlay - 1, -1, -1):
                    b = step - 2 * d
                    if 0 <= b < len(blocks_per[d]):
                        src, dst, l1, l2, bias_col, func, _, resid = layers[d]
                        s0, mc = blocks_per[d][b]
                        conv_block(src, dst, l1, l2, bias_col, func,
                                   s0, mc, resid)

        nc.sync.dma_start(out=ydev[:], in_=yacc)
        nc.sync.dma_start(out=nsdev[:], in_=nsacc)

        for p in (psum_o, psum, pcpool, spool, tpool, dpool, epool, gpool,
                  xpool, wpool):
            p.release()

    _legalize_waits(nc)
    return nc


def _stack_l1l2(Wl):
    # Wl [64o, ic, 3, 3] -> L1, L2 [128, 3, 128]
    ic = Wl.shape[1]
    L1 = np.zeros((128, 3, 128), np.float32)
    L2 = np.zeros((128, 3, 128), np.float32)
    for dx in range(3):
        L1[0:ic, dx, 0:64] = Wl[:, :, 1, dx].T
        L1[64:64 + ic, dx, 0:64] = Wl[:, :, 0, dx].T
        L1[0:ic, dx, 64:128] = Wl[:, :, 0, dx].T
        L2[64:64 + ic, dx, 0:64] = Wl[:, :, 2, dx].T
        L2[0:ic, dx, 64:128] = Wl[:, :, 2, dx].T
        L2[64:64 + ic, dx, 64:128] = Wl[:, :, 1, dx].T
    return L1, L2


def _prep_weights(w_in, w1s, w2s, w_out, flip):
    if flip:
        w_in = w_in[:, :, ::-1, :]
        w1s = w1s[:, :, :, ::-1, :]
        w2s = w2s[:, :, :, ::-1, :]
    l1_in, l2_in = _stack_l1l2(w_in)
    L1m = np.zeros((NMID, 128, 3, 128), np.float32)
    L2m = np.zeros((NMID, 128, 3, 128), np.float32)
    for rb in range(NB):
        L1m[2 * rb], L2m[2 * rb] = _stack_l1l2(w1s[rb])
        L1m[2 * rb + 1], L2m[2 * rb + 1] = _stack_l1l2(w2s[rb])
    # partition-major for the single prefetch DMA: [128, NMID, 3, 128]
    W1m = np.ascontiguousarray(np.transpose(L1m, (1, 0, 2, 3)))
    W2m = np.ascontiguousarray(np.transpose(L2m, (1, 0, 2, 3)))
    wo = w_out[:, :, 0, 0]  # [442, 64]
    if flip:
        # device patch view pairs tap index t*21+u with xe row 4yl+t, which
        # for flipped cores holds xp[... - t]: reverse the t-order of the
        # 441 kernel rows (noise channel 441 stays put).
        idx = np.arange(KK)
        t, u = idx // KSIZE, idx % KSIZE
        perm = (KSIZE - 1 - t) * KSIZE + u
        wo = np.concatenate([wo[perm], wo[KK:]], axis=0)
    wlo = np.zeros((128, NCH), np.float32)
    whi = np.zeros((128, NCH), np.float32)
    wlo[0:64] = wo.T
    whi[64:128] = wo.T
    return l1_in, l2_in, W1m, W2m, wlo, whi


def kernel(x, z, eps, w_in, b_in, w1s, b1s, w2s, b2s, w_out, b_out):
    import ml_dtypes
    x = np.ascontiguousarray(np.asarray(x, np.float32))
    z = np.asarray(z, np.float32)
    eps = np.asarray(eps, np.float32)
    w_in = np.asarray(w_in, np.float32)
    b_in = np.asarray(b_in, np.float32)
    w1s = np.asarray(w1s, np.float32)
    b1s = np.asarray(b1s, np.float32)
    w2s = np.asarray(w2s, np.float32)
    b2s = np.asarray(b2s, np.float32)
    w_out = np.asarray(w_out, np.float32)
    b_out = np.asarray(b_out, np.float32)

    bout_zero = bool(np.all(b_out == 0))
    _enable_ldw_opt()
    key = (bout_zero,)
    if key not in _cache:
        _cache[key] = _build_nc(bout_zero)
    nc = _cache[key]

    weights = {}
    for flip in (False, True):
        weights[flip] = _prep_weights(w_in, w1s, w2s, w_out, flip)

    biases = np.zeros((128, NMID + 1), np.float32)
    biases[0:64, 0] = b_in
    biases[64:128, 0] = b_in
    for rb in range(NB):
        biases[0:64, 1 + 2 * rb] = b1s[rb]
        biases[64:128, 1 + 2 * rb] = b1s[rb]
        biases[0:64, 2 + 2 * rb] = b2s[rb]
        biases[64:128, 2 + 2 * rb] = b2s[rb]
    bout_row = np.ascontiguousarray(b_out.reshape(1, NCH))
    ones_row = np.ones((1, 128), np.float32)

    cols = 4 * np.arange(128)[:, None] + np.arange(KSIZE)[None, :]
    in_maps = []
    for core in range(N_CORES):
        b, half = core // 2, core % 2
        flip = half == 1
        zl = z[b] if not flip else z[b, :, ::-1]
        zg_e = np.zeros((IN_NC, J, X), np.float32)
        zg_o = np.zeros((IN_NC, J, X), np.float32)
        zg_e[:, 1:42, 1:129] = zl[:, 0:81:2]   # rows 0,2,..,80 -> slots 1..41
        zg_o[:, 2:42, 1:129] = zl[:, 1:80:2]   # rows 1,3,..,79 -> slots 2..41
        # horizontally-expanded patch rows
        xp = np.zeros((IN_NC, H + 20, W + 20), dtype=ml_dtypes.bfloat16)
        xp[:, 10:10 + H, 10:10 + W] = x[b]
        if not flip:
            sub = xp[:, 0:XR - 1]                  # xp rows 0..272
        else:
            sub = xp[:, 528 - np.arange(XR - 1)]   # xp rows 528..256 (flipped)
        subc = sub[:, :, cols]                     # [3, 273, 128, 21]
        xe_arr = np.zeros((128, IN_NC, XR, KSIZE), dtype=ml_dtypes.bfloat16)
        xe_arr[:, :, 0:XR - 1] = np.transpose(subc, (2, 0, 1, 3))
        l1_in, l2_in, W1m, W2m, wlo, whi = weights[flip]
        in_maps.append({
            "zg_e": zg_e, "zg_o": zg_o,
            "wl1_in": l1_in, "wl2_in": l2_in,
            "w1m": W1m, "w2m": W2m,
            "wout_lo": wlo, "wout_hi": whi,
            "biases": biases, "bout_r": bout_row, "ones_r": ones_row,
            "xe": xe_arr,
        })

    trace = bool(globals().get("TRACE", False))
    res = run_bass_kernel_spmd(nc, in_maps, core_ids=list(range(N_CORES)),
                               trace=trace)
    globals()["_last_result"] = res

    out = np.zeros((B, IN_NC, h, w), np.float32)
    for bb in range(B):
        ns_sum = (float(res.results[2 * bb]["nsdev"].sum())
                  + float(res.results[2 * bb + 1]["nsdev"].sum()))
        mean_ns = ns_sum / (h * w)
        for half in range(2):
            ydev = res.results[2 * bb + half]["ydev"]  # [128, 3, 64]
            yt = np.transpose(ydev, (1, 2, 0))         # [3, 64, 128]
            if half == 0:
                out[bb, :, 0:64, :] = yt
            else:
                out[bb, :, 64:128, :] = yt[:, ::-1, :]
        out[bb] += mean_ns * eps[bb]
    return out
